# revision 1
# baseline (speedup 1.0000x reference)
"""Trainium2 Bass kernel for nn_BaselineTargetHead (per-sample dynamic MLP).

Strategy: data-parallel over 8 NeuronCores, 8 samples per core.
Per sample the chain is 5 per-sample linear layers over 64 spatial positions:
  [1024,2048] @ [2048,64] -> sigmoid -> ... -> [1,128] @ [128,64] + b

Device kernel (per core, per sample):
  - weights arrive as pre-transposed fp16 "slabs" laid out exactly as the
    SBUF image [128 part, sum_l (Cin_l/128)*Cout_l cols] so a single large
    contiguous DMA loads a sample's full weight set (double-buffered).
  - matmul: lhsT = W^T tile [128(Cin), 128(Cout)], rhs = activation tile
    [128(Cin), 64(spatial)], accumulate over Cin tiles in PSUM fp32.
  - ScalarE applies bias+sigmoid fused, writing fp16 activation tiles that
    feed the next layer without any transposition.
"""

import numpy as np

import concourse.bass as bass
import concourse.mybir as mybir
import concourse.tile as tile
from concourse.bass_utils import run_bass_kernel_spmd

N_CORES = 8
B = 64
S_PER_CORE = B // N_CORES  # 8 samples per core
HW = 64  # 8x8 spatial positions
DIMS = [2048, 1024, 512, 256, 128, 1]
LAYERS = [(2048, 1024), (1024, 512), (512, 256), (256, 128)]  # (Cin, Cout) of fc1..fc4
W_COLS = sum((ci // 128) * co for ci, co in LAYERS)  # 21760 fp16 cols per sample
X_COLS = (2048 // 128) * HW  # 1024
XW5_COLS = X_COLS + 32  # w5 in col X_COLS, zero-padded to 32 cols for a legal M=32 matmul
# bias image columns per sample: fc1 m0..7 | fc2 m0..3 | fc3 m0..1 | fc4 m0 | fc5
BIAS_COL0 = [0, 8, 12, 14]
BIAS_COLS = 16

def _split_ctrl_multiwaits(nc):
    """walrus in this env rejects >1 sync-wait per instruction. Move extra
    waits onto NOPs placed immediately before, on the same engine — engines
    execute in order, so this is semantically identical."""
    n_fixed = 0
    for bb in nc.main_func.blocks:
        insts = bb.instructions
        i = 0
        while i < len(insts):
            ins = insts[i]
            si = ins.sync_info
            if si is not None and si.on_wait and len(si.on_wait) > 1:
                waits = list(si.on_wait)
                new_nops = []
                for j, w in enumerate(waits[1:]):
                    nop = mybir.InstNoOp(name=f"{ins.name}-splitw-{j}", ins=[], outs=[])
                    nop.engine = ins.engine
                    nop.sync_info = mybir.SyncInfo(on_update=[], on_wait=[w])
                    new_nops.append(nop)
                si.on_wait = [waits[0]]
                insts[i:i] = new_nops
                i += len(new_nops)
                n_fixed += 1
            i += 1
    return n_fixed


def _build_nc():
    f16 = mybir.dt.float16
    f32 = mybir.dt.float32
    nc = bass.Bass()
    wslab_d = nc.dram_tensor("wslab", [S_PER_CORE, 128, W_COLS], f16, kind="ExternalInput")
    xw5_d = nc.dram_tensor("xw5", [128, S_PER_CORE * XW5_COLS], f16, kind="ExternalInput")
    bias_d = nc.dram_tensor("bias", [128, S_PER_CORE * BIAS_COLS], f32, kind="ExternalInput")
    out_d = nc.dram_tensor("out", [S_PER_CORE, HW], f32, kind="ExternalOutput")

    sig = mybir.ActivationFunctionType.Sigmoid
    ident = mybir.ActivationFunctionType.Identity

    # L1 weights in slab part A, L2-L4 in part B (separate tiles let layer-1
    # matmuls start before the whole slab has landed)
    A_COLS = (LAYERS[0][0] // 128) * LAYERS[0][1]  # 16384
    B_COLS = W_COLS - A_COLS  # 5376

    with tile.TileContext(nc) as tc:
        with (
            tc.tile_pool(name="wpool", bufs=3) as wpool,
            tc.tile_pool(name="qpool", bufs=2) as qpool,
            tc.tile_pool(name="misc", bufs=1) as misc,
            tc.tile_pool(name="psum", bufs=6, space="PSUM") as psum_pool,
        ):
            # small inputs: one DMA each, issued on the ACT HWDGE queue so the
            # SP queue carries nothing but the big weight-slab stream
            bias_sb = misc.tile([128, S_PER_CORE * BIAS_COLS], f32)
            nc.scalar.dma_start(bias_sb[:], bias_d[:])
            xw5_sb = misc.tile([128, S_PER_CORE * XW5_COLS], f16)
            nc.scalar.dma_start(xw5_sb[:], xw5_d[:])

            HA = A_COLS // 2
            for s in range(S_PER_CORE):
                wta1 = wpool.tile([128, HA], f16, tag="wslabA1")
                nc.sync.dma_start(wta1[:], wslab_d[s, :, 0:HA])
                wta2 = wpool.tile([128, HA], f16, tag="wslabA2")
                nc.sync.dma_start(wta2[:], wslab_d[s, :, HA:A_COLS])
                wtb = wpool.tile([128, B_COLS], f16, tag="wslabB")
                nc.sync.dma_start(wtb[:], wslab_d[s, :, A_COLS:W_COLS])

                xt = xw5_sb[:, s * XW5_COLS : (s + 1) * XW5_COLS]
                q_prev = xt[:, 0:X_COLS]
                # per-layer column offset within its slab tile (A holds L1,
                # B holds L2..L4 back to back)
                layer_off = [0, 0]
                for cin, cout in LAYERS[1:-1]:
                    layer_off.append(layer_off[-1] + (cin // 128) * cout)
                for li, (cin, cout) in enumerate(LAYERS):
                    kt, mt = cin // 128, cout // 128
                    off = layer_off[li]
                    qn = qpool.tile([128, mt * HW], f16, tag=f"q{li}")
                    for m in range(mt):
                        ps = psum_pool.tile([128, HW], f32, tag="ps")
                        for k in range(kt):
                            if li == 0:
                                col = k * cout + m * 128
                                wt, wcol = (wta1, col) if col < HA else (wta2, col - HA)
                            else:
                                wt, wcol = wtb, off + k * cout + m * 128
                            lhsT = wt[:, wcol : wcol + 128]
                            rhs = q_prev[:, k * HW : (k + 1) * HW]
                            nc.tensor.matmul(
                                ps[:], lhsT, rhs, start=(k == 0), stop=(k == kt - 1)
                            )
                        bcol = s * BIAS_COLS + BIAS_COL0[li] + m
                        nc.scalar.activation(
                            qn[:, m * HW : (m + 1) * HW],
                            ps[:],
                            sig,
                            bias=bias_sb[:, bcol : bcol + 1],
                            scale=1.0,
                        )
                    q_prev = qn[:]

                ps5 = psum_pool.tile([128, HW], f32, tag="ps", name="ps5")
                nc.tensor.matmul(
                    ps5[0:32, :], xt[:, X_COLS:XW5_COLS], q_prev[:, 0:HW], start=True, stop=True
                )
                b5col = s * BIAS_COLS + 15
                ot5 = qpool.tile([128, HW], f32, tag="ot5", name="ot5")
                nc.scalar.activation(
                    ot5[:], ps5[:], ident, bias=bias_sb[:, b5col : b5col + 1], scale=1.0
                )
                nc.scalar.dma_start(out_d[s : s + 1, :], ot5[0:1, :])

    _split_ctrl_multiwaits(nc)
    return nc


_NC_CACHE = None


def _get_nc():
    global _NC_CACHE
    if _NC_CACHE is None:
        _NC_CACHE = _build_nc()
    return _NC_CACHE


def _prep_core(inputs, c):
    """Build the per-core input map (numpy only, host-side layout prep)."""
    sl = slice(c * S_PER_CORE, (c + 1) * S_PER_CORE)

    wparts = []
    for li, (cin, cout) in enumerate(LAYERS):
        w = inputs[f"target_fc{li + 1}w"][sl, :, :, 0, 0]  # [S, Cout, Cin]
        # -> [S, 128, (Cin/128)*Cout] with img[s, p, k*Cout+co] = w[s, co, k*128+p]
        wt = w.transpose(0, 2, 1).reshape(S_PER_CORE, cin // 128, 128, cout)
        wt = wt.transpose(0, 2, 1, 3).reshape(S_PER_CORE, 128, -1)
        wparts.append(wt.astype(np.float16))
    wslab = np.ascontiguousarray(np.concatenate(wparts, axis=2))

    x = inputs["target_in_vec"][sl].reshape(S_PER_CORE, 2048 // 128, 128, HW)
    ximg = x.transpose(0, 2, 1, 3).reshape(S_PER_CORE, 128, X_COLS).astype(np.float16)
    w5 = inputs["target_fc5w"][sl, 0, :, 0, 0].astype(np.float16)  # [S, 128]
    w5pad = np.zeros((S_PER_CORE, 128, 32), np.float16)
    w5pad[:, :, 0] = w5
    # partition-major [128, S*XW5_COLS] so the DMA is one big 2D copy
    xw5 = np.ascontiguousarray(
        np.concatenate([ximg, w5pad], axis=2).transpose(1, 0, 2).reshape(128, -1)
    )

    bias = np.zeros((S_PER_CORE, 128, BIAS_COLS), np.float32)
    for li, (cin, cout) in enumerate(LAYERS):
        b = inputs[f"target_fc{li + 1}b"][sl]  # [S, Cout]
        bias[:, :, BIAS_COL0[li] : BIAS_COL0[li] + cout // 128] = b.reshape(
            S_PER_CORE, cout // 128, 128
        ).transpose(0, 2, 1)
    bias[:, 0, 15] = inputs["target_fc5b"][sl, 0]
    bias = np.ascontiguousarray(bias.transpose(1, 0, 2).reshape(128, -1))

    return {"wslab": wslab, "xw5": xw5, "bias": bias}


def kernel(**inputs):
    inputs = {k: np.asarray(v) for k, v in inputs.items()}
    nc = _get_nc()
    in_maps = [_prep_core(inputs, c) for c in range(N_CORES)]
    res = run_bass_kernel_spmd(nc, in_maps, list(range(N_CORES)))
    out = np.concatenate([np.asarray(res.results[c]["out"]) for c in range(N_CORES)], axis=0)
    return out.reshape(B, 8, 8).astype(np.float32)



# revision 3
# speedup vs baseline: 1.5708x; 1.5708x over previous
"""Trainium2 Bass kernel for nn_BaselineTargetHead (per-sample dynamic MLP).

Strategy: data-parallel over 8 NeuronCores, 8 samples per core.
Per sample the chain is 5 per-sample linear layers over 64 spatial positions:
  [1024,2048] @ [2048,64] -> sigmoid -> ... -> [1,128] @ [128,64] + b

The kernel is HBM-bandwidth bound on per-sample weight traffic, so fc1-fc3
weights (98.8% of bytes) and the input x ship as fp8 e3m4 (4 mantissa bits).
Host pre-scales weights by 64 (x by 2) to center N(0,0.02) data in e3m4's
normal range; the inverse scale folds into the ScalarE activation's `scale`.
fc4/fc5 weights stay fp16: the output is a 128-term dot product with no
downstream averaging, so late-layer quantization dominates the error budget
(measured: quantizing w4/w5 costs 1e-3/1.4e-2 rel err, w1-w3 costs nothing).

Device kernel (per core, per sample):
  - weights arrive as pre-transposed "slabs" laid out exactly as the SBUF
    image [128 part, sum_l (Cin_l/128)*Cout_l cols] so a few large contiguous
    DMAs load a sample's full weight set (double-buffered).
  - matmul: lhsT = W^T tile [128(Cin), 128(Cout)] fp8, rhs = activation tile
    [128(Cin), 64(spatial)] fp16, accumulate over Cin tiles in PSUM fp32.
    fp8 stationary weights also halve LDWEIGHTS time via FWL.
  - ScalarE applies scale+bias+sigmoid fused, writing fp16 activation tiles
    that feed the next layer without any transposition.
"""

import numpy as np
import ml_dtypes

import concourse.bass as bass
import concourse.mybir as mybir
import concourse.tile as tile
from concourse.bass_utils import run_bass_kernel_spmd

N_CORES = 8
B = 64
S_PER_CORE = B // N_CORES  # 8 samples per core
HW = 64  # 8x8 spatial positions
DIMS = [2048, 1024, 512, 256, 128, 1]
LAYERS = [(2048, 1024), (1024, 512), (512, 256), (256, 128)]  # (Cin, Cout) of fc1..fc4
N_FP8_LAYERS = 3  # fc1..fc3 weights in e3m4; fc4 stays fp16
W_SCALE_FP8 = 64.0  # host multiplies fp8 weights by this; kernel divides back
X_SCALE_FP8 = 2.0  # same for the input x image
W8_COLS = sum((ci // 128) * co for ci, co in LAYERS[:N_FP8_LAYERS])  # 21504
W16_COLS = sum((ci // 128) * co for ci, co in LAYERS[N_FP8_LAYERS:])  # 256
X_COLS = (2048 // 128) * HW  # 1024
W5_COLS = 32  # w5 zero-padded to 32 cols for a legal M=32 matmul
# bias image columns per sample: fc1 m0..7 | fc2 m0..3 | fc3 m0..1 | fc4 m0 | fc5
BIAS_COL0 = [0, 8, 12, 14]
BIAS_COLS = 16
# per-layer PSUM scale to undo the host-side fp8 pre-scaling
ACT_SCALE = [1.0 / (W_SCALE_FP8 * X_SCALE_FP8), 1.0 / W_SCALE_FP8, 1.0 / W_SCALE_FP8, 1.0]


def _split_ctrl_multiwaits(nc):
    """walrus in this env rejects >1 sync-wait per instruction. Move extra
    waits onto NOPs placed immediately before, on the same engine — engines
    execute in order, so this is semantically identical."""
    n_fixed = 0
    for bb in nc.main_func.blocks:
        insts = bb.instructions
        i = 0
        while i < len(insts):
            ins = insts[i]
            si = ins.sync_info
            if si is not None and si.on_wait and len(si.on_wait) > 1:
                waits = list(si.on_wait)
                new_nops = []
                for j, w in enumerate(waits[1:]):
                    nop = mybir.InstNoOp(name=f"{ins.name}-splitw-{j}", ins=[], outs=[])
                    nop.engine = ins.engine
                    nop.sync_info = mybir.SyncInfo(on_update=[], on_wait=[w])
                    new_nops.append(nop)
                si.on_wait = [waits[0]]
                insts[i:i] = new_nops
                i += len(new_nops)
                n_fixed += 1
            i += 1
    return n_fixed


def _build_nc():
    f8 = mybir.dt.float8e3
    f16 = mybir.dt.float16
    f32 = mybir.dt.float32
    nc = bass.Bass()
    wslab8_d = nc.dram_tensor("wslab8", [S_PER_CORE, 128, W8_COLS], f8, kind="ExternalInput")
    wslab16_d = nc.dram_tensor("wslab16", [S_PER_CORE, 128, W16_COLS], f16, kind="ExternalInput")
    ximg_d = nc.dram_tensor("ximg", [128, S_PER_CORE * X_COLS], f8, kind="ExternalInput")
    w5img_d = nc.dram_tensor("w5img", [128, S_PER_CORE * W5_COLS], f16, kind="ExternalInput")
    bias_d = nc.dram_tensor("bias", [128, S_PER_CORE * BIAS_COLS], f32, kind="ExternalInput")
    out_d = nc.dram_tensor("out", [S_PER_CORE, HW], f32, kind="ExternalOutput")

    sig = mybir.ActivationFunctionType.Sigmoid
    ident = mybir.ActivationFunctionType.Identity

    # fc1 weights split in half (A1/A2) so layer-1 matmuls start before the
    # whole slab has landed; B carries fc2+fc3
    A_COLS = (LAYERS[0][0] // 128) * LAYERS[0][1]  # 16384
    B_COLS = W8_COLS - A_COLS  # 5120
    HA = A_COLS // 2

    with tile.TileContext(nc) as tc:
        with (
            tc.tile_pool(name="wpool", bufs=3) as wpool,
            tc.tile_pool(name="qpool", bufs=2) as qpool,
            tc.tile_pool(name="misc", bufs=1) as misc,
            tc.tile_pool(name="psum", bufs=6, space="PSUM") as psum_pool,
        ):
            # small inputs: one DMA each, issued on the ACT HWDGE queue so the
            # SP queue carries nothing but the big weight-slab stream
            bias_sb = misc.tile([128, S_PER_CORE * BIAS_COLS], f32)
            nc.scalar.dma_start(bias_sb[:], bias_d[:])
            x_sb = misc.tile([128, S_PER_CORE * X_COLS], f8)
            nc.scalar.dma_start(x_sb[:], ximg_d[:])
            w5_sb = misc.tile([128, S_PER_CORE * W5_COLS], f16)
            nc.scalar.dma_start(w5_sb[:], w5img_d[:])

            for s in range(S_PER_CORE):
                wta1 = wpool.tile([128, HA], f8, tag="wslabA1")
                nc.sync.dma_start(wta1[:], wslab8_d[s, :, 0:HA])
                wta2 = wpool.tile([128, HA], f8, tag="wslabA2")
                nc.sync.dma_start(wta2[:], wslab8_d[s, :, HA:A_COLS])
                wtb = wpool.tile([128, B_COLS], f8, tag="wslabB")
                nc.sync.dma_start(wtb[:], wslab8_d[s, :, A_COLS:W8_COLS])
                wtc = wpool.tile([128, W16_COLS], f16, tag="wslabC")
                nc.sync.dma_start(wtc[:], wslab16_d[s, :, :])

                q_prev = x_sb[:, s * X_COLS : (s + 1) * X_COLS]
                # per-layer column offset within its slab tile (A holds fc1,
                # B holds fc2+fc3, C holds fc4)
                layer_off = [0, 0]
                for cin, cout in LAYERS[1:-1]:
                    layer_off.append(layer_off[-1] + (cin // 128) * cout)
                for li, (cin, cout) in enumerate(LAYERS):
                    kt, mt = cin // 128, cout // 128
                    off = layer_off[li]
                    qn = qpool.tile([128, mt * HW], f16, tag=f"q{li}")
                    for m in range(mt):
                        ps = psum_pool.tile([128, HW], f32, tag="ps")
                        for k in range(kt):
                            if li == 0:
                                col = k * cout + m * 128
                                wt, wcol = (wta1, col) if col < HA else (wta2, col - HA)
                            elif li < N_FP8_LAYERS:
                                wt, wcol = wtb, off + k * cout + m * 128
                            else:
                                wt, wcol = wtc, k * cout + m * 128
                            lhsT = wt[:, wcol : wcol + 128]
                            rhs = q_prev[:, k * HW : (k + 1) * HW]
                            nc.tensor.matmul(
                                ps[:], lhsT, rhs, start=(k == 0), stop=(k == kt - 1)
                            )
                        bcol = s * BIAS_COLS + BIAS_COL0[li] + m
                        nc.scalar.activation(
                            qn[:, m * HW : (m + 1) * HW],
                            ps[:],
                            sig,
                            bias=bias_sb[:, bcol : bcol + 1],
                            scale=ACT_SCALE[li],
                        )
                    q_prev = qn[:]

                ps5 = psum_pool.tile([128, HW], f32, tag="ps", name="ps5")
                w5t = w5_sb[:, s * W5_COLS : (s + 1) * W5_COLS]
                nc.tensor.matmul(
                    ps5[0:32, :], w5t, q_prev[:, 0:HW], start=True, stop=True
                )
                b5col = s * BIAS_COLS + 15
                ot5 = qpool.tile([128, HW], f32, tag="ot5", name="ot5")
                nc.scalar.activation(
                    ot5[:], ps5[:], ident, bias=bias_sb[:, b5col : b5col + 1], scale=1.0
                )
                nc.scalar.dma_start(out_d[s : s + 1, :], ot5[0:1, :])

    _split_ctrl_multiwaits(nc)
    return nc


_NC_CACHE = None


def _get_nc():
    global _NC_CACHE
    if _NC_CACHE is None:
        _NC_CACHE = _build_nc()
    return _NC_CACHE


def _to_e3m4(a, scale):
    return np.clip(a * scale, -14.0, 14.0).astype(ml_dtypes.float8_e3m4)


def _prep_core(inputs, c):
    """Build the per-core input map (numpy only, host-side layout prep)."""
    sl = slice(c * S_PER_CORE, (c + 1) * S_PER_CORE)

    def wimg(li):
        cin, cout = LAYERS[li]
        w = inputs[f"target_fc{li + 1}w"][sl, :, :, 0, 0]  # [S, Cout, Cin]
        # -> [S, 128, (Cin/128)*Cout] with img[s, p, k*Cout+co] = w[s, co, k*128+p]
        wt = w.transpose(0, 2, 1).reshape(S_PER_CORE, cin // 128, 128, cout)
        return wt.transpose(0, 2, 1, 3).reshape(S_PER_CORE, 128, -1)

    wslab8 = np.ascontiguousarray(
        _to_e3m4(np.concatenate([wimg(li) for li in range(N_FP8_LAYERS)], axis=2), W_SCALE_FP8)
    )
    wslab16 = np.ascontiguousarray(
        np.concatenate([wimg(li) for li in range(N_FP8_LAYERS, len(LAYERS))], axis=2)
    ).astype(np.float16)

    x = inputs["target_in_vec"][sl].reshape(S_PER_CORE, 2048 // 128, 128, HW)
    ximg = x.transpose(2, 0, 1, 3).reshape(128, S_PER_CORE * X_COLS)
    ximg = np.ascontiguousarray(_to_e3m4(ximg, X_SCALE_FP8))

    w5 = inputs["target_fc5w"][sl, 0, :, 0, 0].astype(np.float16)  # [S, 128]
    w5img = np.zeros((128, S_PER_CORE, W5_COLS), np.float16)
    w5img[:, :, 0] = w5.T
    w5img = np.ascontiguousarray(w5img.reshape(128, -1))

    bias = np.zeros((S_PER_CORE, 128, BIAS_COLS), np.float32)
    for li, (cin, cout) in enumerate(LAYERS):
        b = inputs[f"target_fc{li + 1}b"][sl]  # [S, Cout]
        bias[:, :, BIAS_COL0[li] : BIAS_COL0[li] + cout // 128] = b.reshape(
            S_PER_CORE, cout // 128, 128
        ).transpose(0, 2, 1)
    bias[:, 0, 15] = inputs["target_fc5b"][sl, 0]
    bias = np.ascontiguousarray(bias.transpose(1, 0, 2).reshape(128, -1))

    return {"wslab8": wslab8, "wslab16": wslab16, "ximg": ximg, "w5img": w5img, "bias": bias}


def kernel(**inputs):
    inputs = {k: np.asarray(v) for k, v in inputs.items()}
    nc = _get_nc()
    in_maps = [_prep_core(inputs, c) for c in range(N_CORES)]
    res = run_bass_kernel_spmd(nc, in_maps, list(range(N_CORES)))
    out = np.concatenate([np.asarray(res.results[c]["out"]) for c in range(N_CORES)], axis=0)
    return out.reshape(B, 8, 8).astype(np.float32)
